# revision 2
# baseline (speedup 1.0000x reference)
"""GCN-3 (gnn_message_passing) Trainium2 kernel, 8-core SPMD.

Strategy (dest-node sharded, dense-adjacency spmm, fp8 streams):
  - Nodes (rows of x / destination rows of the spmm) are sharded across the
    8 cores: core k owns nodes [k*1024, (k+1)*1024).
  - The sparse adjacency is densified on the host into A[dest, src] (fp32
    scatter-add, so duplicate edges accumulate exactly like segment_sum),
    then each core receives its slice A[k-slice, :].T as float8_e3m4 with a
    per-slice power-of-2 scale (dequant folded into the spmm combine
    matrices), pre-swizzled p-major so every DMA descriptor is a contiguous
    multi-KB run.  e3m4 (4 mantissa bits) keeps the end-to-end error at
    ~5e-3 while halving both the HBM read and the SBUF footprint vs uint8
    cast-loads (which doubled the SBUF write stream).
  - x ships pre-transposed (feature-major) in float8_e3m4.  The layer-1
    support t1 = x @ W1 runs with W1 tiles STATIONARY (64 cols, alternating
    even/odd feature tiles in the two PE column-group halves) and x
    STREAMING as the moving operand — this removes the 512 x-tile
    LDWEIGHTS loads that made the support matmul weight-load bound.  The
    two column-group partial sums are combined AND transposed to node-major
    in one selection-matrix matmul per node block.
  - Per layer: t is AllGather'd (bf16, tiny); the spmm o = A_k @ T runs as
    a dense matmul with T-tiles stationary (bf16) and the resident A_k.T
    (e3m4) streaming in four concurrent PE column groups; partials are
    summed with a selection-matrix matmul that also applies the A dequant.
  - A tiny AllGather is issued at kernel start so the collective stack's
    communicator-init / entry barrier overlaps the input DMA phase instead
    of stalling the first real AllGather.
  - x slabs stream on the sync HWDGE ring; the adjacency streams on the
    scalar HWDGE ring, paced one slab behind x so x keeps priority.
  - log_softmax runs in fp32 on the owned rows with a single Exp and a
    single Ln activation; the final contraction with Wlin happens on-device
    per core; the 8 partial [8]-vectors are summed on the host (+ blin).
"""
import numpy as np
import ml_dtypes

try:
    import concourse.bass as bass  # noqa: F401
except ImportError:  # pragma: no cover
    import sys

    sys.path.insert(0, "/opt/trn_rl_repo")

import concourse.bacc as bacc
import concourse.tile as tile
import concourse.mybir as mybir
from concourse.bass_utils import run_bass_kernel_spmd

BF16 = ml_dtypes.bfloat16
E3M4 = ml_dtypes.float8_e3m4
N = 8192
NHID = 64
NCLASS = 8
NCORES = 8
SH = N // NCORES          # 1024 nodes per core
NB = SH // 128            # 8 node blocks per core
FT = N // 128             # 64 feature tiles
ST = N // 128             # 64 source tiles
XSL = 8                   # feature tiles per x slab DMA (1 MB)
NSL = FT // XSL           # 8 x slabs
AT_CH = 8                 # source tiles per adjacency chunk DMA (1 MB)

_compiled = None


def _build():
    dt = mybir.dt
    nc = bacc.Bacc("TRN2", target_bir_lowering=False, debug=False, num_devices=NCORES)

    xTr = nc.dram_tensor("xTr", [128, FT, SH], dt.float8e3, kind="ExternalInput")
    ATr = nc.dram_tensor("ATr", [128, ST, SH], dt.float8e3, kind="ExternalInput")
    W1r = nc.dram_tensor("W1r", [128, FT, NHID], dt.bfloat16, kind="ExternalInput")
    W2 = nc.dram_tensor("W2", [NHID, NHID], dt.bfloat16, kind="ExternalInput")
    W3 = nc.dram_tensor("W3", [NHID, NCLASS], dt.bfloat16, kind="ExternalInput")
    b1 = nc.dram_tensor("b1", [NHID, 1], dt.float32, kind="ExternalInput")
    b2 = nc.dram_tensor("b2", [NHID, 1], dt.float32, kind="ExternalInput")
    b3 = nc.dram_tensor("b3", [NCLASS, 1], dt.float32, kind="ExternalInput")
    wl = nc.dram_tensor("wl", [128, NB], dt.float32, kind="ExternalInput")
    id8 = nc.dram_tensor("id8", [NCLASS, NCLASS], dt.float32, kind="ExternalInput")
    s64 = nc.dram_tensor("s64", [128, NHID], dt.bfloat16, kind="ExternalInput")
    s8 = nc.dram_tensor("s8", [128, NCLASS], dt.bfloat16, kind="ExternalInput")
    s64p = nc.dram_tensor("s64p", [128, NHID], dt.bfloat16, kind="ExternalInput")
    y_out = nc.dram_tensor("y", [NCLASS, 1], dt.float32, kind="ExternalOutput")

    AF = mybir.ActivationFunctionType
    ALU = mybir.AluOpType
    rg = [list(range(NCORES))]

    with tile.TileContext(nc) as tc:
        with (
            tc.tile_pool(name="const", bufs=1) as const,
            tc.tile_pool(name="big", bufs=1) as big,
            tc.tile_pool(name="slabs", bufs=3) as slabs,
            tc.tile_pool(name="work", bufs=2) as work,
            tc.tile_pool(name="psum", bufs=8, space="PSUM") as psum,
            tc.tile_pool(name="dram", bufs=1, space="DRAM") as dram,
        ):
            # ---- dummy collective: absorbs the collective stack's
            # communicator-init / entry barrier while input DMAs stream ----
            dum_sb = work.tile([1, 64], dt.bfloat16, tag="dum", name="dum_sb")
            nc.vector.memset(dum_sb[:], 0.0)
            dum_in = dram.tile([1, 64], dt.bfloat16, name="dum_in")
            dum_out = dram.tile(
                [NCORES, 64], dt.bfloat16, addr_space="Shared", name="dum_out"
            )
            nc.gpsimd.dma_start(dum_in[:], dum_sb[:])
            nc.gpsimd.collective_compute(
                "AllGather",
                mybir.AluOpType.bypass,
                replica_groups=rg,
                ins=[dum_in.opt()],
                outs=[dum_out.opt()],
            )

            gp_warm = work.tile([128, 16], dt.float32, tag="gpw", name="gp_warm")
            nc.vector.memset(gp_warm[:], 0.0)
            # preload the Exp/Ln activation tables (1.3us each) while idle so
            # the log_softmax tail doesn't pay them on the critical path
            nc.scalar.activation(gp_warm[:, 0:1], gp_warm[:, 1:2], AF.Exp)
            nc.scalar.activation(gp_warm[:, 2:3], gp_warm[:, 0:1], AF.Ln)

            # ---- constants (small, lead the scalar ring) ----
            W2_sb = const.tile([NHID, NHID], dt.bfloat16)
            nc.scalar.dma_start(W2_sb[:], W2[:])
            W3_sb = const.tile([NHID, NCLASS], dt.bfloat16)
            nc.scalar.dma_start(W3_sb[:], W3[:])
            b1_sb = const.tile([NHID, 1], dt.float32)
            nc.scalar.dma_start(b1_sb[:], b1[:])
            b2_sb = const.tile([NHID, 1], dt.float32)
            nc.scalar.dma_start(b2_sb[:], b2[:])
            b3_sb = const.tile([NCLASS, 1], dt.float32)
            nc.scalar.dma_start(b3_sb[:], b3[:])
            wl_sb = const.tile([128, NB], dt.float32)
            nc.scalar.dma_start(wl_sb[:], wl[:])
            id8_sb = const.tile([NCLASS, NCLASS], dt.float32)
            nc.scalar.dma_start(id8_sb[:], id8[:])
            s64_sb = const.tile([128, NHID], dt.bfloat16)
            nc.scalar.dma_start(s64_sb[:], s64[:])
            s8_sb = const.tile([128, NCLASS], dt.bfloat16)
            nc.scalar.dma_start(s8_sb[:], s8[:])
            s64p_sb = const.tile([128, NHID], dt.bfloat16)
            nc.scalar.dma_start(s64p_sb[:], s64p[:])

            # W1 leads the sync ring so the first slab's matmuls can start
            W1_sb = const.tile([128, FT, NHID], dt.bfloat16)
            nc.sync.dma_start(W1_sb[:], W1r[:])

            AT_sb = big.tile([128, ST, SH], dt.float8e3)

            def load_at_chunk(g):
                nc.scalar.dma_start(
                    AT_sb[:, g * AT_CH:(g + 1) * AT_CH, :],
                    ATr[:, g * AT_CH:(g + 1) * AT_CH, :],
                )

            # ---- layer 1 support: t1 = x_k @ W1.  W1 feature tiles are the
            # stationary operand (even fts in PE cols 0-63, odd fts in cols
            # 64-127); x streams as the moving operand, 512 nodes per matmul.
            # The two column-group partials land in disjoint PSUM partition
            # halves and are summed+transposed per node block afterwards. ----
            ps_l1 = [
                psum.tile([128, 512], dt.float32, tag="ps", name=f"l1p{c}")
                for c in range(2)
            ]
            slab_tiles = []
            for g in range(NSL):
                slab = slabs.tile([128, XSL, SH], dt.float8e3, name="slab", tag="slab")
                slab_tiles.append(slab)
                nc.sync.dma_start(slab[:], xTr[:, g * XSL:(g + 1) * XSL, :])
                for j in range(XSL):
                    f = g * XSL + j
                    grp = (f % 2) * 64
                    for c in range(2):
                        nc.tensor.matmul(
                            ps_l1[c][grp:grp + 64, :],
                            W1_sb[:, f, :],
                            slab[:, j, c * 512:(c + 1) * 512],
                            start=(f < 2),
                            stop=(f >= FT - 2),
                            tile_position=(0, grp),
                            skip_group_check=True,
                        )
            # adjacency chunks paced one slab behind the x stream
            for g in range(ST // AT_CH):
                pace = slab_tiles[min(g + 1, NSL - 1)]
                nc.vector.tensor_copy(
                    AT_sb[0:1, g * AT_CH:g * AT_CH + 1, 0:1], pace[0:1, 0, 0:1],
                )
                load_at_chunk(g)

            # combine + transpose: t1[node, hid] = even + odd partials
            p_bf = [
                work.tile([128, 512], dt.bfloat16, tag="pbf1", name=f"pbf1{c}")
                for c in range(2)
            ]
            for c in range(2):
                nc.vector.tensor_copy(p_bf[c][:], ps_l1[c][:])
            t1n_ps = psum.tile([128, NB, NHID], dt.float32, tag="ps", name="t1n")
            for nb in range(NB):
                c = nb // 4
                nc.tensor.matmul(
                    t1n_ps[:, nb, :],
                    p_bf[c][:, (nb % 4) * 128:(nb % 4 + 1) * 128],
                    s64p_sb[:],
                    start=True,
                    stop=True,
                )
            t1_sb = big.tile([128, NB, NHID], dt.bfloat16, tag="tloc", bufs=2, name="t1_sb")
            nc.vector.tensor_copy(t1_sb[:], t1n_ps[:])

            def allgather(t_sb, width, tag):
                """t_sb [128, NB*width] bf16 -> T_sb [128, NCORES, NB, width]."""
                bounce = dram.tile([128, NB * width], dt.bfloat16, name=f"bounce{tag}")
                gath = dram.tile(
                    [NCORES * 128, NB * width], dt.bfloat16,
                    addr_space="Shared", name=f"gath{tag}",
                )
                nc.gpsimd.dma_start(bounce[:], t_sb[:])
                nc.gpsimd.collective_compute(
                    "AllGather",
                    mybir.AluOpType.bypass,
                    replica_groups=rg,
                    ins=[bounce.opt()],
                    outs=[gath.opt()],
                )
                half = NCORES // 2
                gv = gath[:].rearrange("(r p) (nb h) -> p r nb h", p=128, nb=NB)
                T_a = big.tile(
                    [128, half, NB, width], dt.bfloat16,
                    tag="Tga", bufs=2, name=f"Ta{tag}",
                )
                T_b = big.tile(
                    [128, half, NB, width], dt.bfloat16,
                    tag="Tgb", bufs=2, name=f"Tb{tag}",
                )
                nc.scalar.dma_start(T_a[:], gv[:, :half])
                nc.scalar.dma_start(T_b[:], gv[:, half:])
                return (T_a, T_b)

            def spmm(T_pair, width, bias_sb, relu, out_dt, S_sb, tag,
                     post_chunk=None):
                T_half = lambda st: T_pair[st // (ST // 2)]
                """o.T = sum_st T[st]-stationary @ AT[st]-moving, 4-way col-tiled.

                width=64: two source tiles x two 32-wide output halves run
                concurrently in the four PE column groups. width=8: four
                source tiles. Partials are summed by a selection-matrix
                matmul (which also applies the A dequant scale); DVE applies
                bias (+relu) from PSUM. st-outer order so the adjacency
                stream is consumed progressively.
                """
                h_sb = big.tile([width, SH], out_dt, name=f"h{tag}")
                o_ps = [
                    psum.tile([128, 512], dt.float32, tag="ps", name=f"o{tag}{c}")
                    for c in range(2)
                ]
                ngrp = 2 if width == 64 else 4
                cstep = 128 // ngrp
                rounds = ST // ngrp
                for r in range(rounds):
                    for c in range(2):
                        for j in range(ngrp):
                            st = r * ngrp + j
                            ts_ = T_half(st)
                            nc.tensor.matmul(
                                o_ps[c][j * cstep:j * cstep + width, :],
                                ts_[:, (st // NB) % 4, st % NB, :],
                                AT_sb[:, st, c * 512:(c + 1) * 512],
                                start=(r == 0),
                                stop=(r == rounds - 1),
                                tile_position=(0, j * cstep),
                                skip_group_check=True,
                            )
                for c in range(2):
                    p_bf = work.tile([128, 512], dt.bfloat16, tag="pbf", name=f"pbf{tag}{c}")
                    if ngrp * width == 128:
                        nc.vector.tensor_copy(p_bf[:], o_ps[c][:])
                    else:
                        # unwritten PSUM partitions may hold NaN garbage from a
                        # prior NEFF; zero-fill and copy only the written rows
                        nc.gpsimd.memset(p_bf[:], 0.0)
                        for j in range(ngrp):
                            nc.vector.tensor_copy(
                                p_bf[j * cstep:j * cstep + width, :],
                                o_ps[c][j * cstep:j * cstep + width, :],
                            )
                    comb_ps = psum.tile([width, 512], dt.float32, tag="ps", name=f"cb{tag}{c}")
                    nc.tensor.matmul(comb_ps[:], S_sb[:], p_bf[:], start=True, stop=True)
                    if relu:
                        nc.vector.tensor_scalar(
                            h_sb[:, c * 512:(c + 1) * 512], comb_ps[:],
                            scalar1=bias_sb[:], scalar2=0.0,
                            op0=ALU.add, op1=ALU.max,
                        )
                    else:
                        nc.vector.tensor_scalar_add(
                            h_sb[:, c * 512:(c + 1) * 512], comb_ps[:], bias_sb[:],
                        )
                    if post_chunk is not None:
                        post_chunk(c, h_sb)
                return h_sb

            T1_sb = allgather(t1_sb[:].rearrange("p a b -> p (a b)"), NHID, "1")
            h1_sb = spmm(T1_sb, NHID, b1_sb, True, dt.bfloat16, s64_sb, "1")

            # ---- layer 2 ----
            t2_sb = big.tile([128, NB, NHID], dt.bfloat16, tag="tloc", bufs=2, name="t2_sb")
            for nb in range(NB):
                t2_ps = psum.tile([128, NHID], dt.float32, tag="ps", name=f"t2p{nb}")
                nc.tensor.matmul(
                    t2_ps[:], h1_sb[:, nb * 128:(nb + 1) * 128], W2_sb[:],
                    start=True, stop=True,
                )
                nc.vector.tensor_copy(t2_sb[:, nb, :], t2_ps[:])
            T2_sb = allgather(t2_sb[:].rearrange("p a b -> p (a b)"), NHID, "2")
            h2_sb = spmm(T2_sb, NHID, b2_sb, True, dt.bfloat16, s64_sb, "2")

            # ---- layer 3 ----
            t3_sb = big.tile([128, NB, NCLASS], dt.bfloat16, tag="tloc", bufs=2, name="t3_sb")
            for nb in range(NB):
                t3_ps = psum.tile([128, NCLASS], dt.float32, tag="ps", name=f"t3p{nb}")
                nc.tensor.matmul(
                    t3_ps[:], h2_sb[:, nb * 128:(nb + 1) * 128], W3_sb[:],
                    start=True, stop=True,
                )
                nc.vector.tensor_copy(t3_sb[:, nb, :], t3_ps[:])
            # ---- log_softmax (fp32): per node-block transpose + max + sub
            # interleaved with spmm3's chunks, then one Exp / one Ln ----
            h3n_all = big.tile([128, NB, NCLASS], dt.float32, name="h3n_all")
            mx_all = big.tile([128, NB], dt.float32, name="mx_all")
            sub_all = big.tile([128, NB, NCLASS], dt.float32, name="sub_all")

            def lsm_blocks(c, h_sb):
                nbs = range(c * NB // 2, (c + 1) * NB // 2)
                tr_ps = psum.tile([128, NB // 2, NCLASS], dt.float32, tag="ps", name=f"tr{c}")
                for i, nb in enumerate(nbs):
                    nc.tensor.matmul(
                        tr_ps[:, i, :], h_sb[:, nb * 128:(nb + 1) * 128], id8_sb[:],
                        is_transpose=True, skip_group_check=True,
                    )
                lo = c * NB // 2
                nc.vector.tensor_copy(h3n_all[:, lo:lo + NB // 2, :], tr_ps[:])
                nc.vector.reduce_max(
                    mx_all[:, lo:lo + NB // 2], h3n_all[:, lo:lo + NB // 2, :],
                    axis=mybir.AxisListType.X,
                )
                for nb in nbs:
                    nc.vector.tensor_scalar_sub(
                        sub_all[:, nb, :], h3n_all[:, nb, :], mx_all[:, nb:nb + 1],
                    )

            T3_sb = allgather(t3_sb[:].rearrange("p a b -> p (a b)"), NCLASS, "3")
            h3_sb = spmm(T3_sb, NCLASS, b3_sb, False, dt.float32, s8_sb, "3",
                         post_chunk=lsm_blocks)
            e_all = big.tile([128, NB, NCLASS], dt.float32, name="e_all")
            nc.scalar.activation(
                e_all[:].rearrange("p a b -> p (a b)"),
                sub_all[:].rearrange("p a b -> p (a b)"), AF.Exp,
            )
            esum_all = big.tile([128, NB], dt.float32, name="esum_all")
            nc.vector.reduce_sum(esum_all[:], e_all[:], axis=mybir.AxisListType.X)
            logz_all = big.tile([128, NB], dt.float32, name="logz_all")
            nc.scalar.activation(logz_all[:], esum_all[:], AF.Ln)
            lsm_sb = big.tile([128, NB, NCLASS], dt.float32, name="lsm_sb")
            for nb in range(NB):
                nc.vector.tensor_scalar_sub(
                    lsm_sb[:, nb, :], sub_all[:, nb, :], logz_all[:, nb:nb + 1],
                )

            y_ps = psum.tile([NCLASS, 1], dt.float32, tag="ps", name="y_ps")
            for nb in range(NB):
                nc.tensor.matmul(
                    y_ps[:], lsm_sb[:, nb, :], wl_sb[:, nb:nb + 1],
                    start=(nb == 0), stop=(nb == NB - 1),
                )
            y_sb = work.tile([NCLASS, 1], dt.float32, tag="y", name="y_sb")
            nc.vector.tensor_copy(y_sb[:], y_ps[:])
            nc.scalar.dma_start(y_out[:], y_sb[:])

    nc.compile()
    return nc


def _prep_inputs(x, adj_row, adj_col, adj_val, W1, b1, W2, b2, W3, b3, Wlin):
    import scipy.sparse as sp

    A = sp.coo_matrix(
        (np.asarray(adj_val, np.float32),
         (np.asarray(adj_row, np.int64), np.asarray(adj_col, np.int64))),
        shape=(N, N),
    ).toarray().astype(np.float32)

    W1r = np.ascontiguousarray(
        np.asarray(W1, np.float32).reshape(FT, 128, NHID).transpose(1, 0, 2)
    ).astype(BF16)
    p = np.arange(128)
    s64_mask = (p[:, None] % 64 == np.arange(NHID)[None, :])
    s8_mask = (p[:, None] % 32 == np.arange(NCLASS)[None, :])
    shared = {
        "W1r": W1r,
        "W2": np.asarray(W2, np.float32).astype(BF16),
        "W3": np.asarray(W3, np.float32).astype(BF16),
        "b1": np.ascontiguousarray(np.asarray(b1, np.float32).reshape(NHID, 1)),
        "b2": np.ascontiguousarray(np.asarray(b2, np.float32).reshape(NHID, 1)),
        "b3": np.ascontiguousarray(np.asarray(b3, np.float32).reshape(NCLASS, 1)),
        "id8": np.eye(NCLASS, dtype=np.float32),
        "s64p": s64_mask.astype(BF16),
    }
    x = np.asarray(x, np.float32)
    wlin = np.asarray(Wlin, np.float32)[0]
    in_maps = []
    for k in range(NCORES):
        sl = slice(k * SH, (k + 1) * SH)
        xTk = np.ascontiguousarray(
            x[sl, :].T.reshape(FT, 128, SH).transpose(1, 0, 2)
        ).astype(E3M4)
        Ak = A[sl, :]
        # power-of-2 scale puts the slice in e3m4's normal range ((6, 12]
        # for the max); the exact reciprocal dequant is baked into the
        # selection matrices
        mx = np.float32(Ak.max())
        sq = np.float32(2.0 ** np.floor(np.log2(12.0 / max(mx, np.float32(1e-30)))))
        inv = np.float32(1.0) / sq
        ATk = np.ascontiguousarray(
            (Ak.T * sq).reshape(ST, 128, SH).transpose(1, 0, 2)
        ).astype(E3M4)
        wlk = np.ascontiguousarray(wlin[sl].reshape(NB, 128).T)
        in_maps.append({
            "xTr": xTk, "ATr": ATk, "wl": wlk,
            "s64": (s64_mask * inv).astype(BF16),
            "s8": (s8_mask * inv).astype(BF16),
            **shared,
        })
    return in_maps


def kernel(x, adj_row, adj_col, adj_val, W1, b1, W2, b2, W3, b3, Wlin, blin,
           _trace=False):
    global _compiled
    if _compiled is None:
        _compiled = _build()
    in_maps = _prep_inputs(x, adj_row, adj_col, adj_val, W1, b1, W2, b2, W3, b3, Wlin)
    res = run_bass_kernel_spmd(
        _compiled, in_maps, core_ids=list(range(NCORES)), trace=_trace,
    )
    y = np.zeros(NCLASS, np.float64)
    for k in range(NCORES):
        y += res.results[k]["y"][:, 0].astype(np.float64)
    out = (y + np.asarray(blin, np.float64)[0]).astype(np.float32)[None, :]
    if _trace:
        kernel.last_exec_time_ns = res.exec_time_ns
        kernel.last_profile_json = res.profile_json
        kernel.last_trace = res.instructions_and_trace
    return out


# revision 9
# speedup vs baseline: 1.3126x; 1.3126x over previous
"""GCN-3 (gnn_message_passing) Trainium2 kernel, 8-core SPMD.

Strategy (dest-node sharded, dense-adjacency spmm, fp8 streams):
  - Nodes (rows of x / destination rows of the spmm) are sharded across the
    8 cores: core k owns nodes [k*1024, (k+1)*1024).
  - The sparse adjacency is densified on the host into A[dest, src] (fp32
    scatter-add, so duplicate edges accumulate exactly like segment_sum),
    then each core receives its slice A[k-slice, :].T as float8_e3m4 with a
    per-slice power-of-2 scale (dequant folded into the spmm combine
    matrices), pre-swizzled p-major so every DMA descriptor is a contiguous
    multi-KB run.  e3m4 (4 mantissa bits) keeps the end-to-end error at
    ~5e-3 while halving both the HBM read and the SBUF footprint vs uint8
    cast-loads (which doubled the SBUF write stream).
  - x ships pre-transposed (feature-major) in float8_e3m4.  The layer-1
    support t1 = x @ W1 runs with W1 tiles STATIONARY (64 cols, alternating
    even/odd feature tiles in the two PE column-group halves) and x
    STREAMING as the moving operand — this removes the 512 x-tile
    LDWEIGHTS loads that made the support matmul weight-load bound.  The
    two column-group partial sums are combined AND transposed to node-major
    in one selection-matrix matmul per node block.
  - Per layer: t is AllGather'd (bf16, tiny); the spmm o = A_k @ T runs as
    a dense matmul with T-tiles stationary (bf16) and the resident A_k.T
    (e3m4) streaming in four concurrent PE column groups; partials are
    summed with a selection-matrix matmul that also applies the A dequant.
  - A tiny AllGather is issued at kernel start so the collective stack's
    communicator-init / entry barrier overlaps the input DMA phase instead
    of stalling the first real AllGather.
  - x slabs stream on the sync HWDGE ring; the adjacency streams on the
    scalar HWDGE ring, paced one slab behind x so x keeps priority.
  - log_softmax runs in fp32 on the owned rows with a single Exp and a
    single Ln activation; the final contraction with Wlin happens on-device
    per core; the 8 partial [8]-vectors are summed on the host (+ blin).
"""
import numpy as np
import ml_dtypes

try:
    import concourse.bass as bass  # noqa: F401
except ImportError:  # pragma: no cover
    import sys

    sys.path.insert(0, "/opt/trn_rl_repo")

import concourse.bacc as bacc
import concourse.tile as tile
import concourse.mybir as mybir
from concourse.bass_utils import run_bass_kernel_spmd

BF16 = ml_dtypes.bfloat16
E3M4 = ml_dtypes.float8_e3m4
N = 8192
NHID = 64
NCLASS = 8
NCORES = 8
SH = N // NCORES          # 1024 nodes per core
NB = SH // 128            # 8 node blocks per core
FT = N // 128             # 64 feature tiles
ST = N // 128             # 64 source tiles
XSL = 8                   # feature tiles per x slab DMA (1 MB)
NSL = FT // XSL           # 8 x slabs
AT_CH = 8                 # source tiles per adjacency chunk DMA (1 MB)

_compiled = None


def _build():
    dt = mybir.dt
    nc = bacc.Bacc("TRN2", target_bir_lowering=False, debug=False, num_devices=NCORES)

    xTr = nc.dram_tensor("xTr", [128, FT, SH], dt.float8e3, kind="ExternalInput")
    ATr = nc.dram_tensor("ATr", [128, ST, SH], dt.float8e3, kind="ExternalInput")
    W1r = nc.dram_tensor("W1r", [128, FT, NHID], dt.bfloat16, kind="ExternalInput")
    W2 = nc.dram_tensor("W2", [NHID, NHID], dt.bfloat16, kind="ExternalInput")
    W3 = nc.dram_tensor("W3", [NHID, NCLASS], dt.bfloat16, kind="ExternalInput")
    b1 = nc.dram_tensor("b1", [NHID, 1], dt.float32, kind="ExternalInput")
    b2 = nc.dram_tensor("b2", [NHID, 1], dt.float32, kind="ExternalInput")
    b3 = nc.dram_tensor("b3", [NCLASS, 1], dt.float32, kind="ExternalInput")
    wl = nc.dram_tensor("wl", [128, NB], dt.bfloat16, kind="ExternalInput")
    id8 = nc.dram_tensor("id8", [NCLASS, NCLASS], dt.float32, kind="ExternalInput")
    s64 = nc.dram_tensor("s64", [128, NHID], dt.bfloat16, kind="ExternalInput")
    s8 = nc.dram_tensor("s8", [128, NCLASS], dt.bfloat16, kind="ExternalInput")
    s64p = nc.dram_tensor("s64p", [128, NHID], dt.bfloat16, kind="ExternalInput")
    y_out = nc.dram_tensor("y", [NCLASS, 1], dt.float32, kind="ExternalOutput")

    AF = mybir.ActivationFunctionType
    ALU = mybir.AluOpType
    rg = [list(range(NCORES))]

    with tile.TileContext(nc) as tc:
        with (
            tc.tile_pool(name="const", bufs=1) as const,
            tc.tile_pool(name="big", bufs=1) as big,
            tc.tile_pool(name="slabs", bufs=3) as slabs,
            tc.tile_pool(name="work", bufs=2) as work,
            tc.tile_pool(name="psum", bufs=8, space="PSUM") as psum,
            tc.tile_pool(name="dram", bufs=1, space="DRAM") as dram,
        ):
            gp_warm = work.tile([128, 16], dt.float32, tag="gpw", name="gp_warm")
            nc.vector.memset(gp_warm[:], 0.0)
            # preload the Exp/Ln activation tables (1.3us each) while idle so
            # the log_softmax tail doesn't pay them on the critical path
            nc.scalar.activation(gp_warm[:, 0:1], gp_warm[:, 1:2], AF.Exp)
            nc.scalar.activation(gp_warm[:, 2:3], gp_warm[:, 0:1], AF.Ln)

            # ---- constants (small, lead the scalar ring) ----
            W2_sb = const.tile([NHID, NHID], dt.bfloat16)
            nc.scalar.dma_start(W2_sb[:], W2[:])
            W3_sb = const.tile([NHID, NCLASS], dt.bfloat16)
            nc.scalar.dma_start(W3_sb[:], W3[:])
            b1_sb = const.tile([NHID, 1], dt.float32)
            nc.scalar.dma_start(b1_sb[:], b1[:])
            b2_sb = const.tile([NHID, 1], dt.float32)
            nc.scalar.dma_start(b2_sb[:], b2[:])
            b3_sb = const.tile([NCLASS, 1], dt.float32)
            nc.scalar.dma_start(b3_sb[:], b3[:])
            wl_sb = const.tile([128, NB], dt.bfloat16)
            nc.scalar.dma_start(wl_sb[:], wl[:])
            id8_sb = const.tile([NCLASS, NCLASS], dt.float32)
            nc.scalar.dma_start(id8_sb[:], id8[:])
            s64_sb = const.tile([128, NHID], dt.bfloat16)
            nc.scalar.dma_start(s64_sb[:], s64[:])
            s8_sb = const.tile([128, NCLASS], dt.bfloat16)
            nc.scalar.dma_start(s8_sb[:], s8[:])
            s64p_sb = const.tile([128, NHID], dt.bfloat16)
            nc.scalar.dma_start(s64p_sb[:], s64p[:])

            # W1 leads the sync ring so the first slab's matmuls can start
            W1_sb = const.tile([128, FT, NHID], dt.bfloat16)
            nc.sync.dma_start(W1_sb[:], W1r[:])

            AT_sb = big.tile([128, ST, SH], dt.float8e3)

            def load_at_chunk(g):
                nc.scalar.dma_start(
                    AT_sb[:, g * AT_CH:(g + 1) * AT_CH, :],
                    ATr[:, g * AT_CH:(g + 1) * AT_CH, :],
                )

            # ---- layer 1 support: t1 = x_k @ W1.  W1 feature tiles are the
            # stationary operand (even fts in PE cols 0-63, odd fts in cols
            # 64-127); x streams as the moving operand, 512 nodes per matmul.
            # The two column-group partials land in disjoint PSUM partition
            # halves and are summed+transposed per node block afterwards. ----
            ps_l1 = [
                psum.tile([128, 512], dt.float32, tag="ps", name=f"l1p{c}")
                for c in range(2)
            ]
            slab_tiles = []
            for g in range(NSL):
                slab = slabs.tile([128, XSL, SH], dt.float8e3, name="slab", tag="slab")
                slab_tiles.append(slab)
                nc.sync.dma_start(slab[:], xTr[:, g * XSL:(g + 1) * XSL, :])
                for j in range(XSL):
                    f = g * XSL + j
                    grp = (f % 2) * 64
                    for c in range(2):
                        nc.tensor.matmul(
                            ps_l1[c][grp:grp + 64, :],
                            W1_sb[:, f, :],
                            slab[:, j, c * 512:(c + 1) * 512],
                            start=(f < 2),
                            stop=(f >= FT - 2),
                            tile_position=(0, grp),
                            skip_group_check=True,
                        )
            # adjacency chunks held until the x stream is nearly done so the
            # two HWDGE rings don't round-robin x down to half bandwidth; the
            # spmm doesn't need A until well after the first AllGather
            for g in range(ST // AT_CH):
                pace = slab_tiles[min(2 * g + 5, NSL - 1)]
                nc.vector.tensor_copy(
                    AT_sb[0:1, g * AT_CH:g * AT_CH + 1, 0:1], pace[0:1, 0, 0:1],
                )
                load_at_chunk(g)

            # combine + transpose: t1[node, hid] = even + odd partials
            p_bf = [
                work.tile([128, 512], dt.bfloat16, tag="pbf1", name=f"pbf1{c}")
                for c in range(2)
            ]
            for c in range(2):
                nc.vector.tensor_copy(p_bf[c][:], ps_l1[c][:])
            t1n_ps = psum.tile([128, NB, NHID], dt.float32, tag="ps", name="t1n")
            for nb in range(NB):
                c = nb // 4
                nc.tensor.matmul(
                    t1n_ps[:, nb, :],
                    p_bf[c][:, (nb % 4) * 128:(nb % 4 + 1) * 128],
                    s64p_sb[:],
                    start=True,
                    stop=True,
                )
            t1_sb = big.tile([128, NB, NHID], dt.bfloat16, tag="tloc", bufs=2, name="t1_sb")
            nc.vector.tensor_copy(t1_sb[:], t1n_ps[:])

            def allgather(t_sb, width, tag):
                """t_sb [128, NB*width] bf16 -> T_sb [128, NCORES, NB, width]."""
                bounce = dram.tile([128, NB * width], dt.bfloat16, name=f"bounce{tag}")
                gath = dram.tile(
                    [NCORES * 128, NB * width], dt.bfloat16,
                    addr_space="Shared", name=f"gath{tag}",
                )
                nc.gpsimd.dma_start(bounce[:], t_sb[:])
                nc.gpsimd.collective_compute(
                    "AllGather",
                    mybir.AluOpType.bypass,
                    replica_groups=rg,
                    ins=[bounce.opt()],
                    outs=[gath.opt()],
                )
                half = NCORES // 2
                gv = gath[:].rearrange("(r p) (nb h) -> p r nb h", p=128, nb=NB)
                T_a = big.tile(
                    [128, half, NB, width], dt.bfloat16,
                    tag="Tga", bufs=2, name=f"Ta{tag}",
                )
                T_b = big.tile(
                    [128, half, NB, width], dt.bfloat16,
                    tag="Tgb", bufs=2, name=f"Tb{tag}",
                )
                nc.scalar.dma_start(T_a[:], gv[:, :half])
                nc.scalar.dma_start(T_b[:], gv[:, half:])
                return (T_a, T_b)

            # spmm3's partial tiles are zero-filled up front (unwritten PSUM
            # partitions may hold NaN garbage from a prior NEFF and the
            # selection matmul multiplies them by 0.0 -> NaN); doing the
            # memset here keeps it off the spmm3 critical path
            p3_bf = [
                work.tile([128, 512], dt.bfloat16, tag="pbf3", name=f"pbf3{c}")
                for c in range(2)
            ]
            for c in range(2):
                nc.vector.memset(p3_bf[c][:], 0.0)

            def spmm(T_pair, width, bias_sb, relu, out_dt, S_sb, tag,
                     post_chunk=None):
                T_half = lambda st: T_pair[st // (ST // 2)]
                """o.T = sum_st T[st]-stationary @ AT[st]-moving, 4-way col-tiled.

                width=64: two source tiles x two 32-wide output halves run
                concurrently in the four PE column groups. width=8: four
                source tiles. Partials are summed by a selection-matrix
                matmul (which also applies the A dequant scale); DVE applies
                bias (+relu) from PSUM. st-outer order so the adjacency
                stream is consumed progressively.
                """
                h_sb = big.tile([width, SH], out_dt, name=f"h{tag}")
                o_ps = [
                    psum.tile([128, 512], dt.float32, tag="ps", name=f"o{tag}{c}")
                    for c in range(2)
                ]
                ngrp = 2 if width == 64 else 4
                cstep = 128 // ngrp
                rounds = ST // ngrp
                for r in range(rounds):
                    for c in range(2):
                        for j in range(ngrp):
                            st = r * ngrp + j
                            ts_ = T_half(st)
                            nc.tensor.matmul(
                                o_ps[c][j * cstep:j * cstep + width, :],
                                ts_[:, (st // NB) % 4, st % NB, :],
                                AT_sb[:, st, c * 512:(c + 1) * 512],
                                start=(r == 0),
                                stop=(r == rounds - 1),
                                tile_position=(0, j * cstep),
                                skip_group_check=True,
                            )
                for c in range(2):
                    if ngrp * width == 128:
                        p_bf = work.tile([128, 512], dt.bfloat16, tag="pbf", name=f"pbf{tag}{c}")
                        nc.vector.tensor_copy(p_bf[:], o_ps[c][:])
                    else:
                        p_bf = p3_bf[c]
                        for j in range(ngrp):
                            nc.vector.tensor_copy(
                                p_bf[j * cstep:j * cstep + width, :],
                                o_ps[c][j * cstep:j * cstep + width, :],
                            )
                    comb_ps = psum.tile([width, 512], dt.float32, tag="ps", name=f"cb{tag}{c}")
                    nc.tensor.matmul(comb_ps[:], S_sb[:], p_bf[:], start=True, stop=True)
                    if relu:
                        nc.vector.tensor_scalar(
                            h_sb[:, c * 512:(c + 1) * 512], comb_ps[:],
                            scalar1=bias_sb[:], scalar2=0.0,
                            op0=ALU.add, op1=ALU.max,
                        )
                    else:
                        nc.vector.tensor_scalar_add(
                            h_sb[:, c * 512:(c + 1) * 512], comb_ps[:], bias_sb[:],
                        )
                    if post_chunk is not None:
                        post_chunk(c, h_sb)
                return h_sb

            T1_sb = allgather(t1_sb[:].rearrange("p a b -> p (a b)"), NHID, "1")
            h1_sb = spmm(T1_sb, NHID, b1_sb, True, dt.bfloat16, s64_sb, "1")

            # ---- layer 2 ----
            t2_sb = big.tile([128, NB, NHID], dt.bfloat16, tag="tloc", bufs=2, name="t2_sb")
            for nb in range(NB):
                t2_ps = psum.tile([128, NHID], dt.float32, tag="ps", name=f"t2p{nb}")
                nc.tensor.matmul(
                    t2_ps[:], h1_sb[:, nb * 128:(nb + 1) * 128], W2_sb[:],
                    start=True, stop=True,
                )
                nc.vector.tensor_copy(t2_sb[:, nb, :], t2_ps[:])
            T2_sb = allgather(t2_sb[:].rearrange("p a b -> p (a b)"), NHID, "2")
            h2_sb = spmm(T2_sb, NHID, b2_sb, True, dt.bfloat16, s64_sb, "2")

            # ---- layer 3 ----
            t3_sb = big.tile([128, NB, NCLASS], dt.bfloat16, tag="tloc", bufs=2, name="t3_sb")
            for nb in range(NB):
                t3_ps = psum.tile([128, NCLASS], dt.float32, tag="ps", name=f"t3p{nb}")
                nc.tensor.matmul(
                    t3_ps[:], h2_sb[:, nb * 128:(nb + 1) * 128], W3_sb[:],
                    start=True, stop=True,
                )
                nc.vector.tensor_copy(t3_sb[:, nb, :], t3_ps[:])
            # ---- log_softmax (fp32): per node-block transpose + max + sub
            # interleaved with spmm3's chunks, then one Exp / one Ln ----
            h3n_all = big.tile([128, NB, NCLASS], dt.float32, name="h3n_all")
            mx_all = big.tile([128, NB], dt.float32, name="mx_all")
            sub_all = big.tile([128, NB, NCLASS], dt.float32, name="sub_all")

            def lsm_blocks(c, h_sb):
                nbs = range(c * NB // 2, (c + 1) * NB // 2)
                tr_ps = psum.tile([128, NB // 2, NCLASS], dt.float32, tag="ps", name=f"tr{c}")
                for i, nb in enumerate(nbs):
                    nc.tensor.matmul(
                        tr_ps[:, i, :], h_sb[:, nb * 128:(nb + 1) * 128], id8_sb[:],
                        is_transpose=True, skip_group_check=True,
                    )
                lo = c * NB // 2
                nc.vector.tensor_copy(h3n_all[:, lo:lo + NB // 2, :], tr_ps[:])
                nc.vector.reduce_max(
                    mx_all[:, lo:lo + NB // 2], h3n_all[:, lo:lo + NB // 2, :],
                    axis=mybir.AxisListType.X,
                )
                for nb in nbs:
                    nc.vector.tensor_scalar_sub(
                        sub_all[:, nb, :], h3n_all[:, nb, :], mx_all[:, nb:nb + 1],
                    )

            T3_sb = allgather(t3_sb[:].rearrange("p a b -> p (a b)"), NCLASS, "3")
            h3_sb = spmm(T3_sb, NCLASS, b3_sb, False, dt.float32, s8_sb, "3",
                         post_chunk=lsm_blocks)
            e_all = big.tile([128, NB, NCLASS], dt.float32, name="e_all")
            nc.scalar.activation(
                e_all[:].rearrange("p a b -> p (a b)"),
                sub_all[:].rearrange("p a b -> p (a b)"), AF.Exp,
            )
            esum_all = big.tile([128, NB], dt.float32, name="esum_all")
            nc.vector.reduce_sum(esum_all[:], e_all[:], axis=mybir.AxisListType.X)
            logz_all = big.tile([128, NB], dt.float32, name="logz_all")
            nc.scalar.activation(logz_all[:], esum_all[:], AF.Ln)
            # lsm in bf16 (DVE converts on write) so the final contraction
            # avoids the 4-pass fp32 matmul path; error impact ~1e-4
            lsm_sb = big.tile([128, NB, NCLASS], dt.bfloat16, name="lsm_sb")
            for nb in range(NB):
                nc.vector.tensor_scalar_sub(
                    lsm_sb[:, nb, :], sub_all[:, nb, :], logz_all[:, nb:nb + 1],
                )

            y_ps = psum.tile([NCLASS, 1], dt.float32, tag="ps", name="y_ps")
            for nb in range(NB):
                nc.tensor.matmul(
                    y_ps[:], lsm_sb[:, nb, :], wl_sb[:, nb:nb + 1],
                    start=(nb == 0), stop=(nb == NB - 1),
                )
            y_sb = work.tile([NCLASS, 1], dt.float32, tag="y", name="y_sb")
            nc.vector.tensor_copy(y_sb[:], y_ps[:])
            nc.scalar.dma_start(y_out[:], y_sb[:])

    nc.compile()
    return nc


def _prep_inputs(x, adj_row, adj_col, adj_val, W1, b1, W2, b2, W3, b3, Wlin):
    import scipy.sparse as sp

    A = sp.coo_matrix(
        (np.asarray(adj_val, np.float32),
         (np.asarray(adj_row, np.int64), np.asarray(adj_col, np.int64))),
        shape=(N, N),
    ).toarray().astype(np.float32)

    W1r = np.ascontiguousarray(
        np.asarray(W1, np.float32).reshape(FT, 128, NHID).transpose(1, 0, 2)
    ).astype(BF16)
    p = np.arange(128)
    s64_mask = (p[:, None] % 64 == np.arange(NHID)[None, :])
    s8_mask = (p[:, None] % 32 == np.arange(NCLASS)[None, :])
    shared = {
        "W1r": W1r,
        "W2": np.asarray(W2, np.float32).astype(BF16),
        "W3": np.asarray(W3, np.float32).astype(BF16),
        "b1": np.ascontiguousarray(np.asarray(b1, np.float32).reshape(NHID, 1)),
        "b2": np.ascontiguousarray(np.asarray(b2, np.float32).reshape(NHID, 1)),
        "b3": np.ascontiguousarray(np.asarray(b3, np.float32).reshape(NCLASS, 1)),
        "id8": np.eye(NCLASS, dtype=np.float32),
        "s64p": s64_mask.astype(BF16),
    }
    x = np.asarray(x, np.float32)
    wlin = np.asarray(Wlin, np.float32)[0]
    in_maps = []
    for k in range(NCORES):
        sl = slice(k * SH, (k + 1) * SH)
        xTk = np.ascontiguousarray(
            x[sl, :].T.reshape(FT, 128, SH).transpose(1, 0, 2)
        ).astype(E3M4)
        Ak = A[sl, :]
        # power-of-2 scale puts the slice in e3m4's normal range ((6, 12]
        # for the max); the exact reciprocal dequant is baked into the
        # selection matrices
        mx = np.float32(Ak.max())
        sq = np.float32(2.0 ** np.floor(np.log2(12.0 / max(mx, np.float32(1e-30)))))
        inv = np.float32(1.0) / sq
        ATk = np.ascontiguousarray(
            (Ak.T * sq).reshape(ST, 128, SH).transpose(1, 0, 2)
        ).astype(E3M4)
        wlk = np.ascontiguousarray(wlin[sl].reshape(NB, 128).T).astype(BF16)
        in_maps.append({
            "xTr": xTk, "ATr": ATk, "wl": wlk,
            "s64": (s64_mask * inv).astype(BF16),
            "s8": (s8_mask * inv).astype(BF16),
            **shared,
        })
    return in_maps


def kernel(x, adj_row, adj_col, adj_val, W1, b1, W2, b2, W3, b3, Wlin, blin,
           _trace=False):
    global _compiled
    if _compiled is None:
        _compiled = _build()
    in_maps = _prep_inputs(x, adj_row, adj_col, adj_val, W1, b1, W2, b2, W3, b3, Wlin)
    res = run_bass_kernel_spmd(
        _compiled, in_maps, core_ids=list(range(NCORES)), trace=_trace,
    )
    y = np.zeros(NCLASS, np.float64)
    for k in range(NCORES):
        y += res.results[k]["y"][:, 0].astype(np.float64)
    out = (y + np.asarray(blin, np.float64)[0]).astype(np.float32)[None, :]
    if _trace:
        kernel.last_exec_time_ns = res.exec_time_ns
        kernel.last_profile_json = res.profile_json
        kernel.last_trace = res.instructions_and_trace
    return out


# revision 10
# speedup vs baseline: 1.5570x; 1.1862x over previous
"""GCN-3 (gnn_message_passing) Trainium2 kernel, 8-core SPMD.

Strategy (dest-node sharded, dense-adjacency spmm, fp8 streams):
  - Nodes (rows of x / destination rows of the spmm) are sharded across the
    8 cores: core k owns nodes [k*1024, (k+1)*1024).
  - The sparse adjacency is densified on the host into A[dest, src] (fp32
    scatter-add, so duplicate edges accumulate exactly like segment_sum),
    then each core receives its slice A[k-slice, :].T as float8_e3m4 with a
    per-slice power-of-2 scale (dequant folded into the spmm combine
    matrices), pre-swizzled p-major so every DMA descriptor is a contiguous
    multi-KB run.  e3m4 (4 mantissa bits) keeps the end-to-end error at
    ~5e-3 while halving both the HBM read and the SBUF footprint vs uint8
    cast-loads (which doubled the SBUF write stream).
  - x ships pre-transposed (feature-major) in float8_e3m4.  The layer-1
    support t1 = x @ W1 runs with W1 tiles STATIONARY (64 cols, alternating
    even/odd feature tiles in the two PE column-group halves) and x
    STREAMING as the moving operand — this removes the 512 x-tile
    LDWEIGHTS loads that made the support matmul weight-load bound.  The
    two column-group partial sums are combined AND transposed to node-major
    in one selection-matrix matmul per node block.
  - Per layer: t is AllGather'd (bf16, tiny); the spmm o = A_k @ T runs as
    a dense matmul with T-tiles stationary (bf16) and the resident A_k.T
    (e3m4) streaming in four concurrent PE column groups; partials are
    summed with a selection-matrix matmul that also applies the A dequant.
  - A tiny AllGather is issued at kernel start so the collective stack's
    communicator-init / entry barrier overlaps the input DMA phase instead
    of stalling the first real AllGather.
  - x slabs stream on the sync HWDGE ring; the adjacency streams on the
    scalar HWDGE ring, paced one slab behind x so x keeps priority.
  - log_softmax runs in fp32 on the owned rows with a single Exp and a
    single Ln activation; the final contraction with Wlin happens on-device
    per core; the 8 partial [8]-vectors are summed on the host (+ blin).
"""
import numpy as np
import ml_dtypes

try:
    import concourse.bass as bass  # noqa: F401
except ImportError:  # pragma: no cover
    import sys

    sys.path.insert(0, "/opt/trn_rl_repo")

import concourse.bacc as bacc
import concourse.tile as tile
import concourse.mybir as mybir
from concourse.bass_utils import run_bass_kernel_spmd

BF16 = ml_dtypes.bfloat16
E3M4 = ml_dtypes.float8_e3m4
N = 8192
NHID = 64
NCLASS = 8
NCORES = 8
SH = N // NCORES          # 1024 nodes per core
NB = SH // 128            # 8 node blocks per core
FT = N // 128             # 64 feature tiles
ST = N // 128             # 64 source tiles
XSL = 8                   # feature tiles per x slab DMA (1 MB)
NSL = FT // XSL           # 8 x slabs
AT_CH = 8                 # source tiles per adjacency chunk DMA (1 MB)

_compiled = None


def _build():
    dt = mybir.dt
    nc = bacc.Bacc("TRN2", target_bir_lowering=False, debug=False, num_devices=NCORES)

    xTr = nc.dram_tensor("xTr", [128, FT, SH], dt.float8e3, kind="ExternalInput")
    ATr = nc.dram_tensor("ATr", [128, ST, SH], dt.float8e3, kind="ExternalInput")
    W1r = nc.dram_tensor("W1r", [128, FT, NHID], dt.bfloat16, kind="ExternalInput")
    W2 = nc.dram_tensor("W2", [NHID, NHID], dt.bfloat16, kind="ExternalInput")
    W3 = nc.dram_tensor("W3", [NHID, NCLASS], dt.bfloat16, kind="ExternalInput")
    b1 = nc.dram_tensor("b1", [NHID, 1], dt.float32, kind="ExternalInput")
    b2 = nc.dram_tensor("b2", [NHID, 1], dt.float32, kind="ExternalInput")
    b3 = nc.dram_tensor("b3", [NCLASS, 1], dt.float32, kind="ExternalInput")
    wl = nc.dram_tensor("wl", [128, NB], dt.bfloat16, kind="ExternalInput")
    id8 = nc.dram_tensor("id8", [NCLASS, NCLASS], dt.float32, kind="ExternalInput")
    s64a = nc.dram_tensor("s64a", [128, NHID], dt.bfloat16, kind="ExternalInput")
    s64b = nc.dram_tensor("s64b", [128, NHID], dt.bfloat16, kind="ExternalInput")
    s8 = nc.dram_tensor("s8", [128, NCLASS], dt.bfloat16, kind="ExternalInput")
    s64p = nc.dram_tensor("s64p", [128, NHID], dt.bfloat16, kind="ExternalInput")
    y_out = nc.dram_tensor("y", [NCLASS, 1], dt.float32, kind="ExternalOutput")

    AF = mybir.ActivationFunctionType
    ALU = mybir.AluOpType
    rg = [list(range(NCORES))]

    with tile.TileContext(nc) as tc:
        with (
            tc.tile_pool(name="const", bufs=1) as const,
            tc.tile_pool(name="big", bufs=1) as big,
            tc.tile_pool(name="slabs", bufs=3) as slabs,
            tc.tile_pool(name="work", bufs=2) as work,
            tc.tile_pool(name="psum", bufs=8, space="PSUM") as psum,
            tc.tile_pool(name="dram", bufs=1, space="DRAM") as dram,
        ):
            gp_warm = work.tile([128, 16], dt.float32, tag="gpw", name="gp_warm")
            nc.vector.memset(gp_warm[:], 0.0)
            # preload the Exp/Ln activation tables (1.3us each) while idle so
            # the log_softmax tail doesn't pay them on the critical path
            nc.scalar.activation(gp_warm[:, 0:1], gp_warm[:, 1:2], AF.Exp)
            nc.scalar.activation(gp_warm[:, 2:3], gp_warm[:, 0:1], AF.Ln)

            # ---- constants (small, lead the scalar ring) ----
            W2_sb = const.tile([NHID, NHID], dt.bfloat16)
            nc.scalar.dma_start(W2_sb[:], W2[:])
            W3_sb = const.tile([NHID, NCLASS], dt.bfloat16)
            nc.scalar.dma_start(W3_sb[:], W3[:])
            b1_sb = const.tile([NHID, 1], dt.float32)
            nc.scalar.dma_start(b1_sb[:], b1[:])
            b2_sb = const.tile([NHID, 1], dt.float32)
            nc.scalar.dma_start(b2_sb[:], b2[:])
            b3_sb = const.tile([NCLASS, 1], dt.float32)
            nc.scalar.dma_start(b3_sb[:], b3[:])
            wl_sb = const.tile([128, NB], dt.bfloat16)
            nc.scalar.dma_start(wl_sb[:], wl[:])
            id8_sb = const.tile([NCLASS, NCLASS], dt.float32)
            nc.scalar.dma_start(id8_sb[:], id8[:])
            s64a_sb = const.tile([128, NHID], dt.bfloat16)
            nc.scalar.dma_start(s64a_sb[:], s64a[:])
            s64b_sb = const.tile([128, NHID], dt.bfloat16)
            nc.scalar.dma_start(s64b_sb[:], s64b[:])
            s8_sb = const.tile([128, NCLASS], dt.bfloat16)
            nc.scalar.dma_start(s8_sb[:], s8[:])
            s64p_sb = const.tile([128, NHID], dt.bfloat16)
            nc.scalar.dma_start(s64p_sb[:], s64p[:])

            # W1 leads the sync ring so the first slab's matmuls can start
            W1_sb = const.tile([128, FT, NHID], dt.bfloat16)
            nc.sync.dma_start(W1_sb[:], W1r[:])

            AT_sb = big.tile([128, ST, SH], dt.float8e3)

            def load_at_chunk(g):
                nc.scalar.dma_start(
                    AT_sb[:, g * AT_CH:(g + 1) * AT_CH, :],
                    ATr[:, g * AT_CH:(g + 1) * AT_CH, :],
                )

            # ---- layer 1 support: t1 = x_k @ W1.  W1 feature tiles are the
            # stationary operand (even fts in PE cols 0-63, odd fts in cols
            # 64-127); x streams as the moving operand, 512 nodes per matmul.
            # The two column-group partials land in disjoint PSUM partition
            # halves and are summed+transposed per node block afterwards. ----
            ps_l1 = [
                psum.tile([128, 512], dt.float32, tag="ps", name=f"l1p{c}")
                for c in range(2)
            ]
            slab_tiles = []
            for g in range(NSL):
                slab = slabs.tile([128, XSL, SH], dt.float8e3, name="slab", tag="slab")
                slab_tiles.append(slab)
                nc.sync.dma_start(slab[:], xTr[:, g * XSL:(g + 1) * XSL, :])
                for j in range(XSL):
                    f = g * XSL + j
                    grp = (f % 2) * 64
                    for c in range(2):
                        nc.tensor.matmul(
                            ps_l1[c][grp:grp + 64, :],
                            W1_sb[:, f, :],
                            slab[:, j, c * 512:(c + 1) * 512],
                            start=(f < 2),
                            stop=(f >= FT - 2),
                            tile_position=(0, grp),
                            skip_group_check=True,
                        )
            # adjacency chunks held until the x stream is nearly done so the
            # two HWDGE rings don't round-robin x down to half bandwidth; the
            # spmm doesn't need A until well after the first AllGather
            for g in range(ST // AT_CH):
                pace = slab_tiles[NSL - 1]
                nc.vector.tensor_copy(
                    AT_sb[0:1, g * AT_CH:g * AT_CH + 1, 0:1], pace[0:1, 0, 0:1],
                )
                load_at_chunk(g)

            # combine + transpose: t1[node, hid] = even + odd partials
            p_bf = [
                work.tile([128, 512], dt.bfloat16, tag="pbf1", name=f"pbf1{c}")
                for c in range(2)
            ]
            for c in range(2):
                nc.vector.tensor_copy(p_bf[c][:], ps_l1[c][:])
            t1n_ps = psum.tile([128, NB, NHID], dt.float32, tag="ps", name="t1n")
            for nb in range(NB):
                c = nb // 4
                nc.tensor.matmul(
                    t1n_ps[:, nb, :],
                    p_bf[c][:, (nb % 4) * 128:(nb % 4 + 1) * 128],
                    s64p_sb[:],
                    start=True,
                    stop=True,
                )
            # t1 ships on the AllGather wire as e3m4 scaled by 1/4 (max ~7.4);
            # the x4 dequant is folded into the spmm1 selection matrix
            t1_sb = big.tile([128, NB, NHID], dt.float8e3, tag="tloc", bufs=2, name="t1_sb")
            nc.vector.tensor_scalar_mul(t1_sb[:], t1n_ps[:], 0.25)

            def allgather(t_sb, width, tag, dtp):
                """t_sb [128, NB*width] -> T_sb [128, NCORES, NB, width]."""
                bounce = dram.tile([128, NB * width], dtp, name=f"bounce{tag}")
                gath = dram.tile(
                    [NCORES * 128, NB * width], dtp,
                    addr_space="Shared", name=f"gath{tag}",
                )
                nc.gpsimd.dma_start(bounce[:], t_sb[:])
                nc.gpsimd.collective_compute(
                    "AllGather",
                    mybir.AluOpType.bypass,
                    replica_groups=rg,
                    ins=[bounce.opt()],
                    outs=[gath.opt()],
                )
                half = NCORES // 2
                gv = gath[:].rearrange("(r p) (nb h) -> p r nb h", p=128, nb=NB)
                T_a = big.tile(
                    [128, half, NB, width], dtp,
                    tag="Tga", bufs=2, name=f"Ta{tag}",
                )
                T_b = big.tile(
                    [128, half, NB, width], dtp,
                    tag="Tgb", bufs=2, name=f"Tb{tag}",
                )
                # per-core loads so the spmm can start on the first core's
                # tiles while the rest of the gather output is still landing
                for i in range(half):
                    nc.scalar.dma_start(T_a[:, i], gv[:, i])
                for i in range(half):
                    nc.scalar.dma_start(T_b[:, i], gv[:, half + i])
                return (T_a, T_b)

            # spmm3's partial tiles are zero-filled up front (unwritten PSUM
            # partitions may hold NaN garbage from a prior NEFF and the
            # selection matmul multiplies them by 0.0 -> NaN); doing the
            # memset here keeps it off the spmm3 critical path
            p3_bf = [
                work.tile([128, 512], dt.bfloat16, tag="pbf3", name=f"pbf3{c}")
                for c in range(2)
            ]
            for c in range(2):
                nc.vector.memset(p3_bf[c][:], 0.0)

            def spmm(T_pair, width, bias_sb, relu, out_dt, S_sb, tag,
                     post_chunk=None):
                T_half = lambda st: T_pair[st // (ST // 2)]
                """o.T = sum_st T[st]-stationary @ AT[st]-moving, 4-way col-tiled.

                width=64: two source tiles x two 32-wide output halves run
                concurrently in the four PE column groups. width=8: four
                source tiles. Partials are summed by a selection-matrix
                matmul (which also applies the A dequant scale); DVE applies
                bias (+relu) from PSUM. st-outer order so the adjacency
                stream is consumed progressively.
                """
                h_sb = big.tile([width, SH], out_dt, name=f"h{tag}")
                o_ps = [
                    psum.tile([128, 512], dt.float32, tag="ps", name=f"o{tag}{c}")
                    for c in range(2)
                ]
                ngrp = 2 if width == 64 else 4
                cstep = 128 // ngrp
                rounds = ST // ngrp
                for r in range(rounds):
                    for c in range(2):
                        for j in range(ngrp):
                            st = r * ngrp + j
                            ts_ = T_half(st)
                            nc.tensor.matmul(
                                o_ps[c][j * cstep:j * cstep + width, :],
                                ts_[:, (st // NB) % 4, st % NB, :],
                                AT_sb[:, st, c * 512:(c + 1) * 512],
                                start=(r == 0),
                                stop=(r == rounds - 1),
                                tile_position=(0, j * cstep),
                                skip_group_check=True,
                            )
                for c in range(2):
                    if ngrp * width == 128:
                        p_bf = work.tile([128, 512], dt.bfloat16, tag="pbf", name=f"pbf{tag}{c}")
                        nc.vector.tensor_copy(p_bf[:], o_ps[c][:])
                    else:
                        p_bf = p3_bf[c]
                        for j in range(ngrp):
                            nc.vector.tensor_copy(
                                p_bf[j * cstep:j * cstep + width, :],
                                o_ps[c][j * cstep:j * cstep + width, :],
                            )
                    comb_ps = psum.tile([width, 512], dt.float32, tag="ps", name=f"cb{tag}{c}")
                    nc.tensor.matmul(comb_ps[:], S_sb[:], p_bf[:], start=True, stop=True)
                    if relu:
                        nc.vector.tensor_scalar(
                            h_sb[:, c * 512:(c + 1) * 512], comb_ps[:],
                            scalar1=bias_sb[:], scalar2=0.0,
                            op0=ALU.add, op1=ALU.max,
                        )
                    else:
                        nc.vector.tensor_scalar_add(
                            h_sb[:, c * 512:(c + 1) * 512], comb_ps[:], bias_sb[:],
                        )
                    if post_chunk is not None:
                        post_chunk(c, h_sb)
                return h_sb

            T1_sb = allgather(t1_sb[:].rearrange("p a b -> p (a b)"), NHID, "1", dt.float8e3)
            h1_sb = spmm(T1_sb, NHID, b1_sb, True, dt.bfloat16, s64a_sb, "1")

            # ---- layer 2 ----
            t2_sb = big.tile([128, NB, NHID], dt.float8e3, tag="tloc", bufs=2, name="t2_sb")
            for nb in range(NB):
                t2_ps = psum.tile([128, NHID], dt.float32, tag="ps", name=f"t2p{nb}")
                nc.tensor.matmul(
                    t2_ps[:], h1_sb[:, nb * 128:(nb + 1) * 128], W2_sb[:],
                    start=True, stop=True,
                )
                nc.vector.tensor_scalar_mul(t2_sb[:, nb, :], t2_ps[:], 0.125)
            T2_sb = allgather(t2_sb[:].rearrange("p a b -> p (a b)"), NHID, "2", dt.float8e3)
            h2_sb = spmm(T2_sb, NHID, b2_sb, True, dt.bfloat16, s64b_sb, "2")

            # ---- layer 3 ----
            t3_sb = big.tile([128, NB, NCLASS], dt.bfloat16, tag="tloc", bufs=2, name="t3_sb")
            for nb in range(NB):
                t3_ps = psum.tile([128, NCLASS], dt.float32, tag="ps", name=f"t3p{nb}")
                nc.tensor.matmul(
                    t3_ps[:], h2_sb[:, nb * 128:(nb + 1) * 128], W3_sb[:],
                    start=True, stop=True,
                )
                nc.vector.tensor_copy(t3_sb[:, nb, :], t3_ps[:])
            # ---- log_softmax (fp32): per node-block transpose + max + sub
            # interleaved with spmm3's chunks, then one Exp / one Ln ----
            h3n_all = big.tile([128, NB, NCLASS], dt.float32, name="h3n_all")
            mx_all = big.tile([128, NB], dt.float32, name="mx_all")
            sub_all = big.tile([128, NB, NCLASS], dt.float32, name="sub_all")

            def lsm_blocks(c, h_sb):
                nbs = range(c * NB // 2, (c + 1) * NB // 2)
                tr_ps = psum.tile([128, NB // 2, NCLASS], dt.float32, tag="ps", name=f"tr{c}")
                for i, nb in enumerate(nbs):
                    nc.tensor.matmul(
                        tr_ps[:, i, :], h_sb[:, nb * 128:(nb + 1) * 128], id8_sb[:],
                        is_transpose=True, skip_group_check=True,
                    )
                lo = c * NB // 2
                nc.vector.tensor_copy(h3n_all[:, lo:lo + NB // 2, :], tr_ps[:])
                nc.vector.reduce_max(
                    mx_all[:, lo:lo + NB // 2], h3n_all[:, lo:lo + NB // 2, :],
                    axis=mybir.AxisListType.X,
                )
                for nb in nbs:
                    nc.vector.tensor_scalar_sub(
                        sub_all[:, nb, :], h3n_all[:, nb, :], mx_all[:, nb:nb + 1],
                    )

            T3_sb = allgather(t3_sb[:].rearrange("p a b -> p (a b)"), NCLASS, "3", dt.bfloat16)
            h3_sb = spmm(T3_sb, NCLASS, b3_sb, False, dt.float32, s8_sb, "3",
                         post_chunk=lsm_blocks)
            e_all = big.tile([128, NB, NCLASS], dt.float32, name="e_all")
            nc.scalar.activation(
                e_all[:].rearrange("p a b -> p (a b)"),
                sub_all[:].rearrange("p a b -> p (a b)"), AF.Exp,
            )
            esum_all = big.tile([128, NB], dt.float32, name="esum_all")
            nc.vector.reduce_sum(esum_all[:], e_all[:], axis=mybir.AxisListType.X)
            logz_all = big.tile([128, NB], dt.float32, name="logz_all")
            nc.scalar.activation(logz_all[:], esum_all[:], AF.Ln)
            # lsm in bf16 (DVE converts on write) so the final contraction
            # avoids the 4-pass fp32 matmul path; error impact ~1e-4
            lsm_sb = big.tile([128, NB, NCLASS], dt.bfloat16, name="lsm_sb")
            for nb in range(NB):
                nc.vector.tensor_scalar_sub(
                    lsm_sb[:, nb, :], sub_all[:, nb, :], logz_all[:, nb:nb + 1],
                )

            y_ps = psum.tile([NCLASS, 1], dt.float32, tag="ps", name="y_ps")
            for nb in range(NB):
                nc.tensor.matmul(
                    y_ps[:], lsm_sb[:, nb, :], wl_sb[:, nb:nb + 1],
                    start=(nb == 0), stop=(nb == NB - 1),
                )
            y_sb = work.tile([NCLASS, 1], dt.float32, tag="y", name="y_sb")
            nc.vector.tensor_copy(y_sb[:], y_ps[:])
            nc.scalar.dma_start(y_out[:], y_sb[:])

    nc.compile()
    return nc


def _prep_inputs(x, adj_row, adj_col, adj_val, W1, b1, W2, b2, W3, b3, Wlin):
    import scipy.sparse as sp

    A = sp.coo_matrix(
        (np.asarray(adj_val, np.float32),
         (np.asarray(adj_row, np.int64), np.asarray(adj_col, np.int64))),
        shape=(N, N),
    ).toarray().astype(np.float32)

    W1r = np.ascontiguousarray(
        np.asarray(W1, np.float32).reshape(FT, 128, NHID).transpose(1, 0, 2)
    ).astype(BF16)
    p = np.arange(128)
    s64_mask = (p[:, None] % 64 == np.arange(NHID)[None, :])
    s8_mask = (p[:, None] % 32 == np.arange(NCLASS)[None, :])
    shared = {
        "W1r": W1r,
        "W2": np.asarray(W2, np.float32).astype(BF16),
        "W3": np.asarray(W3, np.float32).astype(BF16),
        "b1": np.ascontiguousarray(np.asarray(b1, np.float32).reshape(NHID, 1)),
        "b2": np.ascontiguousarray(np.asarray(b2, np.float32).reshape(NHID, 1)),
        "b3": np.ascontiguousarray(np.asarray(b3, np.float32).reshape(NCLASS, 1)),
        "id8": np.eye(NCLASS, dtype=np.float32),
        "s64p": s64_mask.astype(BF16),
    }
    x = np.asarray(x, np.float32)
    wlin = np.asarray(Wlin, np.float32)[0]
    in_maps = []
    for k in range(NCORES):
        sl = slice(k * SH, (k + 1) * SH)
        xTk = np.ascontiguousarray(
            x[sl, :].T.reshape(FT, 128, SH).transpose(1, 0, 2)
        ).astype(E3M4)
        Ak = A[sl, :]
        # power-of-2 scale puts the slice in e3m4's normal range ((6, 12]
        # for the max); the exact reciprocal dequant is baked into the
        # selection matrices
        mx = np.float32(Ak.max())
        sq = np.float32(2.0 ** np.floor(np.log2(12.0 / max(mx, np.float32(1e-30)))))
        inv = np.float32(1.0) / sq
        ATk = np.ascontiguousarray(
            (Ak.T * sq).reshape(ST, 128, SH).transpose(1, 0, 2)
        ).astype(E3M4)
        wlk = np.ascontiguousarray(wlin[sl].reshape(NB, 128).T).astype(BF16)
        in_maps.append({
            "xTr": xTk, "ATr": ATk, "wl": wlk,
            "s64a": (s64_mask * (inv * np.float32(4.0))).astype(BF16),
            "s64b": (s64_mask * (inv * np.float32(8.0))).astype(BF16),
            "s8": (s8_mask * inv).astype(BF16),
            **shared,
        })
    return in_maps


def kernel(x, adj_row, adj_col, adj_val, W1, b1, W2, b2, W3, b3, Wlin, blin,
           _trace=False):
    global _compiled
    if _compiled is None:
        _compiled = _build()
    in_maps = _prep_inputs(x, adj_row, adj_col, adj_val, W1, b1, W2, b2, W3, b3, Wlin)
    res = run_bass_kernel_spmd(
        _compiled, in_maps, core_ids=list(range(NCORES)), trace=_trace,
    )
    y = np.zeros(NCLASS, np.float64)
    for k in range(NCORES):
        y += res.results[k]["y"][:, 0].astype(np.float64)
    out = (y + np.asarray(blin, np.float64)[0]).astype(np.float32)[None, :]
    if _trace:
        kernel.last_exec_time_ns = res.exec_time_ns
        kernel.last_profile_json = res.profile_json
        kernel.last_trace = res.instructions_and_trace
    return out
